# revision 5
# baseline (speedup 1.0000x reference)
"""GraphTransformerEncoder on 8 trn2 NeuronCores (Bass/Tile).

Sharding: nodes split contiguously across 8 cores (6250 each, padded to
6272 = 49*128), relabeled by in-degree (desc) within a core so each 128-node
group has a tight max in-degree. Edges live on their destination owner core.

Per layer, each core projects K/V for its local nodes (bf16), two AllGathers
build two global KV tables (A = first 3200 local rows of every core, B = the
remaining 3072) so that table row ids fit in int16 for the hardware
`dma_gather` SWDGE instruction (<= 1024 descriptors per gather). The edge
stage gathers source KV rows grid-by-grid (dst-node lanes on partitions,
degree slots on columns), runs segment softmax + aggregation with wide DVE
ops, and a single wide finalize does the gated skip, GELU and LayerNorm.

The edge embedding term (edge_attr @ We) is folded in analytically:
  alpha = q.(k_src+e)/sqrt(C) = q.k_src/sqrt(C) + (q.We/sqrt(C)) * ea
  out_num = sum ex*(v_src+e) = sum ex*v_src + (sum ex*ea) * We_row
so K/V rows are gathered raw and no per-edge embedding is materialized.
"""
import os
import sys
sys.path.insert(0, "/opt/trn_rl_repo")

import numpy as np

import concourse.bass as bass
import concourse.bacc as bacc
import concourse.tile as tile
from concourse import mybir
from concourse import bass_utils
from concourse.library_config import mlp
from concourse.masks import make_identity

N = 50000
E = 800000
IN = 256
HID = 128
H = 4
C = 32
L = 3
P = 128
NCORES = 8
NLOC = N // NCORES            # 6250
NT = (NLOC + P - 1) // P      # 49 node tiles per core
NPAD = NT * P                 # 6272
SPLA = 3200                   # local rows in table A (groups 0..24)
SPLB = NPAD - SPLA            # 3072 (groups 25..48)
GSPL = SPLA // P              # 25 groups in the A half
TABA = NCORES * SPLA          # 25600 rows  (< 32767, int16-safe)
TABB = NCORES * SPLB          # 24576 rows
DCAP = 16                     # max grid columns processed in one DVE round
JC = 8                        # max columns per dma_gather (1024 descriptors)
RSC = 1.0 / np.sqrt(C)

F32 = mybir.dt.float32
BF16 = mybir.dt.bfloat16
I16 = mybir.dt.int16


def _grid_plan(DgA, DgB):
    """Deterministic traversal of edge grids shared by host preprocessing and
    kernel build. Returns (plans, SUMD2) with plans a list of
    (T, g, first, w, c0, gws): table T, dst group g, first-subgrid flag,
    subgrid width w, absolute column offset c0, gather chunk widths gws."""
    plans = []
    co = 0
    for T, Dg in ((0, DgA), (1, DgB)):
        for g in range(NT):
            d = int(Dg[g])
            si = 0
            while d > 0:
                w = min(d, DCAP)
                gws = []
                r = w
                while r > 0:
                    gw = min(r, JC)
                    gws.append(gw)
                    r -= gw
                plans.append((T, g, si == 0, w, co, gws))
                co += w
                si += 1
                d -= w
    return plans, co


def _preprocess(x, edge_index, edge_attr):
    """Host-side sharding. Returns per-core arrays + metadata."""
    src = edge_index[0].astype(np.int64)
    dst = edge_index[1].astype(np.int64)
    ea = edge_attr[:, 0].astype(np.float32)

    deg = np.bincount(dst, minlength=N)
    perm = np.empty((NCORES, NLOC), np.int64)   # new local idx -> orig local idx
    invp = np.empty(N, np.int64)                # orig global -> new local idx
    for c in range(NCORES):
        d = deg[c * NLOC:(c + 1) * NLOC]
        p = np.argsort(-d, kind="stable")
        perm[c] = p
        invp[c * NLOC + p] = np.arange(NLOC)
    new_gid = (np.arange(N) // NLOC) * NPAD + invp

    owner = dst // NLOC
    dst_nl = invp[dst]
    src_core = new_gid[src] // NPAD
    src_loc = new_gid[src] % NPAD
    src_T = (src_loc >= SPLA).astype(np.int64)
    src_row = np.where(src_T == 0, src_core * SPLA + src_loc,
                       src_core * SPLB + (src_loc - SPLA))

    # per-core, per-table, per-node counts (padded nodes have count 0)
    cnts = np.zeros((NCORES, 2, NPAD), np.int64)
    for c in range(NCORES):
        mc = owner == c
        for T in range(2):
            m = mc & (src_T == T)
            cnts[c, T] = np.bincount(dst_nl[m], minlength=NPAD)
    DgA = cnts[:, 0].reshape(NCORES, NT, P).max(axis=2).max(axis=0)
    DgB = cnts[:, 1].reshape(NCORES, NT, P).max(axis=2).max(axis=0)
    offs = np.zeros((2, NT), np.int64)
    offs[0] = np.concatenate([[0], np.cumsum(DgA)[:-1]])
    offs[1] = DgA.sum() + np.concatenate([[0], np.cumsum(DgB)[:-1]])
    SUMD2 = int(DgA.sum() + DgB.sum())

    gidx = np.zeros((NCORES, P, SUMD2), np.int16)
    eav = np.zeros((NCORES, P, SUMD2), np.float32)
    msk = np.zeros((NCORES, P, SUMD2), np.float32)
    for c in range(NCORES):
        mc = owner == c
        for T in range(2):
            m = mc & (src_T == T)
            dl = dst_nl[m]
            sr = src_row[m]
            ev = ea[m]
            order = np.argsort(dl, kind="stable")
            dls = dl[order]
            srs = sr[order]
            eas = ev[order]
            counts = cnts[c, T]
            starts = np.zeros(NPAD, np.int64)
            starts[1:] = np.cumsum(counts)[:-1]
            jidx = np.arange(len(dls)) - starts[dls]
            grp = dls // P
            lane = dls % P
            col = offs[T][grp] + jidx
            gidx[c, lane, col] = srs.astype(np.int16)
            eav[c, lane, col] = eas
            msk[c, lane, col] = 1.0

    # wrapped+replicated int16 index stream for dma_gather, in plan order
    plans, SUMD2b = _grid_plan(DgA, DgB)
    assert SUMD2b == SUMD2
    CHT8 = sum(gw * 8 for (_, _, _, _, _, gws) in plans for gw in gws)
    idxw = np.zeros((NCORES, P, CHT8), np.int16)
    pos = 0
    for (_, _, _, _, c0, gws) in plans:
        cc = c0
        for gw in gws:
            chunk = gidx[:, :, cc:cc + gw]              # [NCORES, 128, gw]
            wrapped = chunk.reshape(NCORES, 8, 16, gw).transpose(0, 2, 3, 1) \
                           .reshape(NCORES, 16, gw * 8)
            idxw[:, :, pos:pos + gw * 8] = np.tile(wrapped, (1, 8, 1))
            cc += gw
            pos += gw * 8
    assert pos == CHT8

    xT = np.zeros((NCORES, IN, NPAD), np.float32)
    for c in range(NCORES):
        xT[c, :, :NLOC] = x[c * NLOC + perm[c]].T

    return xT, idxw, eav, msk, DgA, DgB, perm


_CACHE = {}


def _build(DgA, DgB):
    key = (tuple(DgA), tuple(DgB))
    if key in _CACHE:
        return _CACHE[key]

    plans, SUMD2 = _grid_plan(DgA, DgB)
    CHT8 = sum(gw * 8 for (_, _, _, _, _, gws) in plans for gw in gws)

    nc = bacc.Bacc("TRN2", target_bir_lowering=False, debug=False,
                   num_devices=NCORES)

    # ---- kernel I/O ----
    xT_d = nc.dram_tensor("xT", [IN, NPAD], F32, kind="ExternalInput").ap()
    idxw_d = nc.dram_tensor("idxw", [P, CHT8], I16, kind="ExternalInput").ap()
    eav_d = nc.dram_tensor("eav", [P, SUMD2], F32, kind="ExternalInput").ap()
    msk_d = nc.dram_tensor("msk", [P, SUMD2], F32, kind="ExternalInput").ap()
    Wi_d = nc.dram_tensor("Wi", [IN, HID], F32, kind="ExternalInput").ap()
    bi_d = nc.dram_tensor("bi", [1, HID], F32, kind="ExternalInput").ap()
    Wq_d = nc.dram_tensor("Wq", [L, HID, HID], F32, kind="ExternalInput").ap()
    Wk_d = nc.dram_tensor("Wk", [L, HID, HID], F32, kind="ExternalInput").ap()
    Wv_d = nc.dram_tensor("Wv", [L, HID, HID], F32, kind="ExternalInput").ap()
    Ws_d = nc.dram_tensor("Ws", [L, HID, HID], F32, kind="ExternalInput").ap()
    bq_d = nc.dram_tensor("bq", [L, 1, HID], F32, kind="ExternalInput").ap()
    bk_d = nc.dram_tensor("bk", [L, 1, HID], F32, kind="ExternalInput").ap()
    bv_d = nc.dram_tensor("bv", [L, 1, HID], F32, kind="ExternalInput").ap()
    bs_d = nc.dram_tensor("bs", [L, 1, HID], F32, kind="ExternalInput").ap()
    WeR_d = nc.dram_tensor("WeR", [L, P, HID], F32, kind="ExternalInput").ap()
    We16_d = nc.dram_tensor("We16", [L, P, HID], BF16, kind="ExternalInput").ap()
    wAR_d = nc.dram_tensor("wAR", [L, P, HID], F32, kind="ExternalInput").ap()
    wBR_d = nc.dram_tensor("wBR", [L, P, HID], F32, kind="ExternalInput").ap()
    lgR_d = nc.dram_tensor("lgR", [L, P, HID], F32, kind="ExternalInput").ap()
    lbR_d = nc.dram_tensor("lbR", [L, P, HID], F32, kind="ExternalInput").ap()
    out_d = nc.dram_tensor("out_h", [NPAD, HID], F32, kind="ExternalOutput").ap()

    kv_in = nc.dram_tensor("kv_in", [NPAD, 2 * HID], BF16).ap()
    kv_tabA = nc.dram_tensor("kv_tabA", [TABA, 2 * HID], BF16,
                             addr_space="Shared").ap()
    kv_tabB = nc.dram_tensor("kv_tabB", [TABB, 2 * HID], BF16,
                             addr_space="Shared").ap()

    cc_sem = nc.alloc_semaphore(name="cc_sem")

    # ---- persistent SBUF ----
    h_sb = nc.alloc_sbuf_tensor("h_sb", [P, NPAD], F32).ap()
    s_sb = nc.alloc_sbuf_tensor("s_sb", [P, NPAD], F32).ap()
    q_sb = nc.alloc_sbuf_tensor("q_sb", [P, NPAD], BF16).ap()
    Wi_sb = nc.alloc_sbuf_tensor("Wi_sb", [P, 2 * HID], F32).ap()
    W_sb = nc.alloc_sbuf_tensor("W_sb", [P, 4 * L * HID], F32).ap()
    bias_sb = nc.alloc_sbuf_tensor("bias_sb", [1, (4 * L + 1) * HID], F32).ap()
    rep_sb = nc.alloc_sbuf_tensor("rep_sb", [P, 5 * L * HID], F32).ap()
    rep16_sb = nc.alloc_sbuf_tensor("rep16_sb", [P, L * HID], BF16).ap()
    idxw_sb = nc.alloc_sbuf_tensor("idxw_sb", [P, CHT8], I16).ap()
    eav_sb = nc.alloc_sbuf_tensor("eav_sb", [P, SUMD2], F32).ap()
    msk_sb = nc.alloc_sbuf_tensor("msk_sb", [P, SUMD2], F32).ap()
    qwe_sb = nc.alloc_sbuf_tensor("qwe_sb", [P, NT * H], F32).ap()
    den_sb = nc.alloc_sbuf_tensor("den_sb", [P, NT * H], F32).ap()
    sden_sb = nc.alloc_sbuf_tensor("sden_sb", [P, NT * H], F32).ap()
    num_sb = nc.alloc_sbuf_tensor("num_sb", [P, NT * HID], F32).ap()
    ones_sb = nc.alloc_sbuf_tensor("ones_sb", [1, HID], F32).ap()
    eps_sb = nc.alloc_sbuf_tensor("eps_sb", [P, 1], F32).ap()
    ident = nc.alloc_sbuf_tensor("ident", [P, P], F32).ap()

    def Wslice(kind, l):  # kind: 0=q 1=k 2=v 3=s
        c0 = (l * 4 + kind) * HID
        return W_sb[:, c0:c0 + HID]

    def bslice(kind, l):
        c0 = (l * 4 + kind) * HID
        return bias_sb[:, c0:c0 + HID]

    bi_sl = bias_sb[:, 4 * L * HID:(4 * L + 1) * HID]

    def repslice(kind, l):  # 0=WeR 1=wAR 2=wBR 3=lgR 4=lbR
        c0 = (l * 5 + kind) * HID
        return rep_sb[:, c0:c0 + HID]

    def rep16slice(l):
        return rep16_sb[:, l * HID:(l + 1) * HID]

    def bcast(ap, dims):
        """Insert broadcast/reshape dims: dims is the new AP dim list."""
        return bass.AP(ap.tensor, ap.offset, dims)

    # ================= stage 0: consts + input projection =================
    with tile.TileContext(nc) as tc:
        make_identity(nc, ident)
        nc.vector.memset(ones_sb, 1.0)
        nc.vector.memset(eps_sb, 1e-5)
        nc.gpsimd.load_library(mlp)
        nc.sync.dma_start(out=Wi_sb[:, 0:HID], in_=Wi_d[0:P, :])
        nc.sync.dma_start(out=Wi_sb[:, HID:2 * HID], in_=Wi_d[P:2 * P, :])
        nc.sync.dma_start(out=bi_sl, in_=bi_d[:])
        nc.sync.dma_start(out=idxw_sb[:], in_=idxw_d[:])
        nc.sync.dma_start(out=eav_sb[:], in_=eav_d[:])
        nc.sync.dma_start(out=msk_sb[:], in_=msk_d[:])
        for l in range(L):
            nc.sync.dma_start(out=Wslice(0, l), in_=Wq_d[l])
            nc.sync.dma_start(out=Wslice(1, l), in_=Wk_d[l])
            nc.sync.dma_start(out=Wslice(2, l), in_=Wv_d[l])
            nc.sync.dma_start(out=Wslice(3, l), in_=Ws_d[l])
            nc.sync.dma_start(out=bslice(0, l), in_=bq_d[l])
            nc.sync.dma_start(out=bslice(1, l), in_=bk_d[l])
            nc.sync.dma_start(out=bslice(2, l), in_=bv_d[l])
            nc.sync.dma_start(out=bslice(3, l), in_=bs_d[l])
            nc.sync.dma_start(out=repslice(0, l), in_=WeR_d[l])
            nc.sync.dma_start(out=repslice(1, l), in_=wAR_d[l])
            nc.sync.dma_start(out=repslice(2, l), in_=wBR_d[l])
            nc.sync.dma_start(out=repslice(3, l), in_=lgR_d[l])
            nc.sync.dma_start(out=repslice(4, l), in_=lbR_d[l])
            nc.sync.dma_start(out=rep16slice(l), in_=We16_d[l])
        with tc.tile_pool(name="s0", bufs=3) as pool, \
             tc.tile_pool(name="s0p", bufs=2, space="PSUM") as ppool:
            for t in range(NT):
                cs = slice(t * P, (t + 1) * P)
                xa = pool.tile([P, P], F32)
                xb = pool.tile([P, P], F32)
                nc.sync.dma_start(out=xa[:], in_=xT_d[0:P, cs])
                nc.sync.dma_start(out=xb[:], in_=xT_d[P:2 * P, cs])
                ps = ppool.tile([P, HID], F32, space="PSUM")
                nc.tensor.matmul(out=ps[:], lhsT=xa[:], rhs=Wi_sb[:, 0:HID],
                                 start=True, stop=False)
                nc.tensor.matmul(out=ps[:], lhsT=xb[:], rhs=Wi_sb[:, HID:2 * HID],
                                 start=False, stop=False)
                nc.tensor.matmul(out=ps[:], lhsT=ones_sb, rhs=bi_sl,
                                 start=False, stop=True)
                nc.scalar.copy(out=h_sb[:, cs], in_=ps[:])

    # ================= layers =================
    def proj_groups(l, t0, t1):
        with tile.TileContext(nc) as tc:
            with tc.tile_pool(name=f"A{l}_{t0}", bufs=3) as pool, \
                 tc.tile_pool(name=f"Ap{l}_{t0}", bufs=2, space="PSUM") as ppool:
                for t in range(t0, t1):
                    cs = slice(t * P, (t + 1) * P)
                    pst = ppool.tile([P, P], F32, space="PSUM")
                    nc.tensor.transpose(out=pst[:], in_=h_sb[:, cs], identity=ident)
                    hT = pool.tile([P, P], F32)
                    nc.scalar.copy(out=hT[:], in_=pst[:])

                    pkv = ppool.tile([P, 2 * HID], F32, space="PSUM")
                    nc.tensor.matmul(out=pkv[:, 0:HID], lhsT=hT[:],
                                     rhs=Wslice(1, l), start=True, stop=False)
                    nc.tensor.matmul(out=pkv[:, 0:HID], lhsT=ones_sb,
                                     rhs=bslice(1, l), start=False, stop=True)
                    nc.tensor.matmul(out=pkv[:, HID:2 * HID], lhsT=hT[:],
                                     rhs=Wslice(2, l), start=True, stop=False)
                    nc.tensor.matmul(out=pkv[:, HID:2 * HID], lhsT=ones_sb,
                                     rhs=bslice(2, l), start=False, stop=True)
                    kvt = pool.tile([P, 2 * HID], BF16)
                    nc.scalar.copy(out=kvt[:], in_=pkv[:])
                    nc.sync.dma_start(out=kv_in[t * P:(t + 1) * P, :], in_=kvt[:])

                    pqs = ppool.tile([P, 2 * HID], F32, space="PSUM")
                    nc.tensor.matmul(out=pqs[:, 0:HID], lhsT=hT[:],
                                     rhs=Wslice(0, l), start=True, stop=False)
                    nc.tensor.matmul(out=pqs[:, 0:HID], lhsT=ones_sb,
                                     rhs=bslice(0, l), start=False, stop=True)
                    nc.tensor.matmul(out=pqs[:, HID:2 * HID], lhsT=hT[:],
                                     rhs=Wslice(3, l), start=True, stop=False)
                    nc.tensor.matmul(out=pqs[:, HID:2 * HID], lhsT=ones_sb,
                                     rhs=bslice(3, l), start=False, stop=True)
                    nc.scalar.mul(out=q_sb[:, cs], in_=pqs[:, 0:HID], mul=RSC)
                    nc.scalar.copy(out=s_sb[:, cs], in_=pqs[:, HID:2 * HID])

    def edge_pass(l, T, tab):
        """Process all grids of table T; accumulate into den/sden/num."""
        zero_done = set()
        pos_map = _chunk_positions(plans)
        with tile.TileContext(nc) as tc:
            with tc.tile_pool(name=f"B{l}_{T}", bufs=2) as kpool, \
                 tc.tile_pool(name=f"Bw{l}_{T}", bufs=3) as wpool:
                for pi, (Tp, g, first0, w, c0, gws) in enumerate(plans):
                    if Tp != T:
                        continue
                    first = first0 if T == 0 else (first0 and DgA[g] == 0)
                    gH = slice(g * H, (g + 1) * H)
                    gF = slice(g * HID, (g + 1) * HID)
                    cs = slice(g * P, (g + 1) * P)

                    if T == 0 and first0:
                        # qwe[n,h] = sum_c q[n,hc]*We16[hc]
                        qwe_s = wpool.tile([P, HID], F32)
                        nc.vector.tensor_tensor(out=qwe_s[:], in0=q_sb[:, cs],
                                                in1=rep16slice(l),
                                                op=mybir.AluOpType.mult)
                        nc.vector.tensor_reduce(
                            out=qwe_sb[:, gH],
                            in_=qwe_s[:].rearrange("p (h c) -> p h c", h=H),
                            axis=mybir.AxisListType.X, op=mybir.AluOpType.add)

                    kvg = kpool.tile([P, w, 2 * HID], BF16)
                    cc = 0
                    for gi, gw in enumerate(gws):
                        pos = pos_map[(pi, gi)]
                        nc.gpsimd.dma_gather(
                            kvg[:, cc:cc + gw, :], tab,
                            idxw_sb[:, pos:pos + gw * 8],
                            gw * P, gw * P, 2 * HID)
                        cc += gw

                    kj = kvg[:, :, 0:HID]
                    vj = kvg[:, :, HID:2 * HID]
                    qs = q_sb[:, cs]
                    qb = bcast(qs, [list(qs.ap[0]), [0, w], list(qs.ap[1])])
                    qk = wpool.tile([P, w * HID], BF16)
                    nc.vector.tensor_tensor(
                        out=qk[:].rearrange("p (w f) -> p w f", w=w),
                        in0=kj, in1=qb, op=mybir.AluOpType.mult)
                    alph = wpool.tile([P, w * H], F32)
                    nc.vector.tensor_reduce(
                        out=alph[:],
                        in_=qk[:].rearrange("p (w h c) -> p w h c", w=w, h=H),
                        axis=mybir.AxisListType.X, op=mybir.AluOpType.add)
                    # + qWe*ea
                    eas = eav_sb[:, c0:c0 + w]
                    eab = bcast(eas, [list(eas.ap[0]), list(eas.ap[1]), [0, H]])
                    qwes = qwe_sb[:, gH]
                    qweb = bcast(qwes, [list(qwes.ap[0]), [0, w], list(qwes.ap[1])])
                    term = wpool.tile([P, w * H], F32)
                    nc.vector.tensor_tensor(
                        out=term[:].rearrange("p (w h) -> p w h", w=w),
                        in0=eab, in1=qweb, op=mybir.AluOpType.mult)
                    nc.vector.tensor_tensor(out=alph[:], in0=alph[:],
                                            in1=term[:], op=mybir.AluOpType.add)
                    ex = wpool.tile([P, w * H], F32)
                    nc.scalar.activation(out=ex[:], in_=alph[:],
                                         func=mybir.ActivationFunctionType.Exp)
                    mks = msk_sb[:, c0:c0 + w]
                    mkb = bcast(mks, [list(mks.ap[0]), list(mks.ap[1]), [0, H]])
                    nc.vector.tensor_tensor(
                        out=ex[:].rearrange("p (w h) -> p w h", w=w),
                        in0=ex[:].rearrange("p (w h) -> p w h", w=w),
                        in1=mkb, op=mybir.AluOpType.mult)
                    # den accumulation
                    exT = bcast(ex[:], [list(ex[:].ap[0]), [1, H], [H, w]])
                    if first:
                        nc.vector.tensor_reduce(out=den_sb[:, gH], in_=exT,
                                                axis=mybir.AxisListType.X,
                                                op=mybir.AluOpType.add)
                    else:
                        dpart = wpool.tile([P, H], F32)
                        nc.vector.tensor_reduce(out=dpart[:], in_=exT,
                                                axis=mybir.AxisListType.X,
                                                op=mybir.AluOpType.add)
                        nc.vector.tensor_tensor(out=den_sb[:, gH],
                                                in0=den_sb[:, gH], in1=dpart[:],
                                                op=mybir.AluOpType.add)
                    # sden accumulation (sum ex*ea)
                    t2 = wpool.tile([P, w * H], F32)
                    nc.vector.tensor_tensor(
                        out=t2[:].rearrange("p (w h) -> p w h", w=w),
                        in0=ex[:].rearrange("p (w h) -> p w h", w=w),
                        in1=eab, op=mybir.AluOpType.mult)
                    t2T = bcast(t2[:], [list(t2[:].ap[0]), [1, H], [H, w]])
                    if first:
                        nc.vector.tensor_reduce(out=sden_sb[:, gH], in_=t2T,
                                                axis=mybir.AxisListType.X,
                                                op=mybir.AluOpType.add)
                    else:
                        spart = wpool.tile([P, H], F32)
                        nc.vector.tensor_reduce(out=spart[:], in_=t2T,
                                                axis=mybir.AxisListType.X,
                                                op=mybir.AluOpType.add)
                        nc.vector.tensor_tensor(out=sden_sb[:, gH],
                                                in0=sden_sb[:, gH], in1=spart[:],
                                                op=mybir.AluOpType.add)
                    # num accumulation (sum ex*v)
                    ex16 = wpool.tile([P, w * H], BF16)
                    nc.vector.tensor_copy(out=ex16[:], in_=ex[:])
                    exb = bcast(ex16[:], [list(ex16[:].ap[0]), [H, w], [1, H], [0, C]])
                    exv = wpool.tile([P, w * HID], BF16)
                    nc.vector.tensor_tensor(
                        out=exv[:].rearrange("p (w f) -> p w f", w=w),
                        in0=vj, in1=exb, op=mybir.AluOpType.mult)
                    exvT = bcast(exv[:], [list(exv[:].ap[0]), [1, HID], [HID, w]])
                    if first:
                        nc.vector.tensor_reduce(out=num_sb[:, gF], in_=exvT,
                                                axis=mybir.AxisListType.X,
                                                op=mybir.AluOpType.add)
                    else:
                        npart = wpool.tile([P, HID], F32)
                        nc.vector.tensor_reduce(out=npart[:], in_=exvT,
                                                axis=mybir.AxisListType.X,
                                                op=mybir.AluOpType.add)
                        nc.vector.tensor_tensor(out=num_sb[:, gF],
                                                in0=num_sb[:, gF], in1=npart[:],
                                                op=mybir.AluOpType.add)
                # groups with no edges in either table: zero the accumulators
                if T == 1:
                    for g in range(NT):
                        if DgA[g] == 0 and DgB[g] == 0 and g not in zero_done:
                            zero_done.add(g)
                            nc.vector.memset(den_sb[:, g * H:(g + 1) * H], 0.0)
                            nc.vector.memset(sden_sb[:, g * H:(g + 1) * H], 0.0)
                            nc.vector.memset(num_sb[:, g * HID:(g + 1) * HID], 0.0)

    def finalize(l):
        """Gated skip + GELU + LayerNorm for all groups, quarter-wide passes."""
        with tile.TileContext(nc) as tc:
            with tc.tile_pool(name=f"F{l}", bufs=4) as pool, \
                 tc.tile_pool(name=f"Fs{l}", bufs=2) as spool:
                for G0 in range(0, NT, 13):
                    GN = min(13, NT - G0)
                    W = GN * HID
                    gh = slice(G0 * H, (G0 + GN) * H)
                    gf = slice(G0 * HID, G0 * HID + W)
                    numv = num_sb[:, gf]
                    sv = s_sb[:, gf]
                    hv = h_sb[:, gf]
                    p0 = list(numv.ap[0])

                    def repb(ap):   # [P, HID] -> (p, G, h, c) with G bcast
                        return bcast(ap, [list(ap.ap[0]), [0, GN], [C, H], [1, C]])

                    def repbf(ap):  # [P, HID] -> (p, G, f) with G bcast
                        return bcast(ap, [list(ap.ap[0]), [0, GN], [1, HID]])

                    nc.vector.tensor_scalar_add(den_sb[:, gh], den_sb[:, gh], 1e-16)
                    rden = spool.tile([P, GN * H], F32)
                    nc.vector.reciprocal(out=rden[:], in_=den_sb[:, gh])

                    # num += sden_h * We_row ; num *= rden_h  -> outt (in place)
                    sdv = sden_sb[:, gh]
                    fix = pool.tile([P, W], F32, tag="wide")
                    nc.vector.tensor_tensor(
                        out=fix[:].rearrange("p (G h c) -> p G h c", G=GN, h=H),
                        in0=repb(repslice(0, l)),
                        in1=bcast(sdv, [list(sdv.ap[0]), [H, GN], [1, H], [0, C]]),
                        op=mybir.AluOpType.mult)
                    nc.vector.tensor_tensor(out=numv, in0=numv, in1=fix[:],
                                            op=mybir.AluOpType.add)
                    nc.vector.tensor_tensor(
                        out=numv.rearrange("p (G h c) -> p G h c", G=GN, h=H),
                        in0=numv.rearrange("p (G h c) -> p G h c", G=GN, h=H),
                        in1=bcast(rden[:], [list(rden[:].ap[0]), [H, GN], [1, H], [0, C]]),
                        op=mybir.AluOpType.mult)

                    # beta = sigmoid(outt.wA + s.wB) per (p, G)
                    scr = pool.tile([P, W], F32, tag="wide")
                    nc.vector.tensor_tensor(
                        out=scr[:].rearrange("p (G f) -> p G f", G=GN),
                        in0=numv.rearrange("p (G f) -> p G f", G=GN),
                        in1=repbf(repslice(1, l)), op=mybir.AluOpType.mult)
                    dotA = spool.tile([P, GN], F32)
                    nc.vector.tensor_reduce(
                        out=dotA[:], in_=scr[:].rearrange("p (G f) -> p G f", G=GN),
                        axis=mybir.AxisListType.X, op=mybir.AluOpType.add)
                    scr2 = pool.tile([P, W], F32, tag="wide")
                    nc.vector.tensor_tensor(
                        out=scr2[:].rearrange("p (G f) -> p G f", G=GN),
                        in0=sv.rearrange("p (G f) -> p G f", G=GN),
                        in1=repbf(repslice(2, l)), op=mybir.AluOpType.mult)
                    dotB = spool.tile([P, GN], F32)
                    nc.vector.tensor_reduce(
                        out=dotB[:], in_=scr2[:].rearrange("p (G f) -> p G f", G=GN),
                        axis=mybir.AxisListType.X, op=mybir.AluOpType.add)
                    nc.vector.tensor_tensor(out=dotA[:], in0=dotA[:], in1=dotB[:],
                                            op=mybir.AluOpType.add)
                    bet = spool.tile([P, GN], F32)
                    nc.scalar.activation(out=bet[:], in_=dotA[:],
                                         func=mybir.ActivationFunctionType.Sigmoid)
                    # conv = outt + beta*(s - outt)   (in place on numv)
                    d1 = pool.tile([P, W], F32, tag="wide")
                    nc.vector.tensor_tensor(out=d1[:], in0=sv, in1=numv,
                                            op=mybir.AluOpType.subtract)
                    nc.vector.tensor_tensor(
                        out=d1[:].rearrange("p (G f) -> p G f", G=GN),
                        in0=d1[:].rearrange("p (G f) -> p G f", G=GN),
                        in1=bcast(bet[:], [list(bet[:].ap[0]), [1, GN], [0, HID]]),
                        op=mybir.AluOpType.mult)
                    nc.vector.tensor_tensor(out=numv, in0=numv, in1=d1[:],
                                            op=mybir.AluOpType.add)
                    # gelu + residual
                    gl = pool.tile([P, W], F32, tag="wide")
                    nc.scalar.activation(out=gl[:], in_=numv,
                                         func=mybir.ActivationFunctionType.Gelu)
                    nc.vector.tensor_tensor(out=gl[:], in0=gl[:], in1=hv,
                                            op=mybir.AluOpType.add)
                    # layernorm
                    musum = spool.tile([P, GN], F32)
                    nc.vector.tensor_reduce(
                        out=musum[:], in_=gl[:].rearrange("p (G f) -> p G f", G=GN),
                        axis=mybir.AxisListType.X, op=mybir.AluOpType.add)
                    mu = spool.tile([P, GN], F32)
                    nc.scalar.mul(out=mu[:], in_=musum[:], mul=1.0 / HID)
                    nc.vector.tensor_tensor(
                        out=gl[:].rearrange("p (G f) -> p G f", G=GN),
                        in0=gl[:].rearrange("p (G f) -> p G f", G=GN),
                        in1=bcast(mu[:], [list(mu[:].ap[0]), [1, GN], [0, HID]]),
                        op=mybir.AluOpType.subtract)
                    sq = pool.tile([P, W], F32, tag="wide")
                    nc.vector.tensor_tensor(out=sq[:], in0=gl[:], in1=gl[:],
                                            op=mybir.AluOpType.mult)
                    vsum = spool.tile([P, GN], F32)
                    nc.vector.tensor_reduce(
                        out=vsum[:], in_=sq[:].rearrange("p (G f) -> p G f", G=GN),
                        axis=mybir.AxisListType.X, op=mybir.AluOpType.add)
                    sd = spool.tile([P, GN], F32)
                    nc.scalar.activation(out=sd[:], in_=vsum[:],
                                         func=mybir.ActivationFunctionType.Sqrt,
                                         scale=1.0 / HID, bias=eps_sb)
                    rstd = spool.tile([P, GN], F32)
                    nc.vector.reciprocal(out=rstd[:], in_=sd[:])
                    nc.vector.tensor_tensor(
                        out=gl[:].rearrange("p (G f) -> p G f", G=GN),
                        in0=gl[:].rearrange("p (G f) -> p G f", G=GN),
                        in1=bcast(rstd[:], [list(rstd[:].ap[0]), [1, GN], [0, HID]]),
                        op=mybir.AluOpType.mult)
                    nc.vector.tensor_tensor(
                        out=gl[:].rearrange("p (G f) -> p G f", G=GN),
                        in0=gl[:].rearrange("p (G f) -> p G f", G=GN),
                        in1=repbf(repslice(3, l)), op=mybir.AluOpType.mult)
                    nc.vector.tensor_tensor(
                        out=hv.rearrange("p (G f) -> p G f", G=GN),
                        in0=gl[:].rearrange("p (G f) -> p G f", G=GN),
                        in1=repbf(repslice(4, l)), op=mybir.AluOpType.add)
                    if l == L - 1:
                        oap = bass.AP(out_d.tensor, G0 * P * HID,
                                      [[HID, P], [P * HID, GN], [1, HID]])
                        nc.sync.dma_start(out=oap, in_=hv)

    for l in range(L):
        proj_groups(l, 0, GSPL)
        nc.gpsimd.collective_compute(
            "AllGather", mybir.AluOpType.bypass,
            ins=[kv_in[0:SPLA, :]], outs=[kv_tabA[:]],
            replica_groups=[list(range(NCORES))],
        ).then_inc(cc_sem, 1)
        proj_groups(l, GSPL, NT)
        nc.gpsimd.collective_compute(
            "AllGather", mybir.AluOpType.bypass,
            ins=[kv_in[SPLA:NPAD, :]], outs=[kv_tabB[:]],
            replica_groups=[list(range(NCORES))],
        ).then_inc(cc_sem, 1)
        nc.gpsimd.wait_ge(cc_sem, 2 * l + 1)
        edge_pass(l, 0, kv_tabA[:])
        nc.gpsimd.wait_ge(cc_sem, 2 * l + 2)
        edge_pass(l, 1, kv_tabB[:])
        finalize(l)

    nc.compile()
    _CACHE[(tuple(DgA), tuple(DgB))] = nc
    return nc


def _chunk_positions(plans):
    """Map (plan index, chunk index) -> column offset in idxw (int16 cols)."""
    pos_map = {}
    pos = 0
    for pi, (_, _, _, _, _, gws) in enumerate(plans):
        for gi, gw in enumerate(gws):
            pos_map[(pi, gi)] = pos
            pos += gw * 8
    return pos_map


def _run_sim(nc, in_maps):
    import concourse.bass_interp as bi
    from concourse.bass_interp import MultiCoreSim, Direction
    import scipy.special as sp

    if not getattr(bi.InstructionExecutor, "_gelu_patched", False):
        _orig = bi.InstructionExecutor.visit_InstActivation

        def _patched(self, instruction, *, reg_snapshot=None):
            if instruction.func == mybir.ActivationFunctionType.Gelu:
                instruction.func = mybir.ActivationFunctionType.Identity
                try:
                    res = _orig(self, instruction, reg_snapshot=reg_snapshot)
                finally:
                    instruction.func = mybir.ActivationFunctionType.Gelu
                v = self.view_ap(instruction.outs[0], Direction.WRITE,
                                 instruction, reg_snapshot=reg_snapshot)
                x = np.asarray(v, np.float32)
                v[:] = (0.5 * x * (1.0 + sp.erf(x / np.sqrt(2.0)))).astype(v.dtype)
                return res
            return _orig(self, instruction, reg_snapshot=reg_snapshot)

        bi.InstructionExecutor.visit_InstActivation = _patched
        bi.InstructionExecutor._gelu_patched = True

    sim = MultiCoreSim(nc, num_cores=NCORES)
    for c, core_sim in sim.cores.items():
        for k, v in in_maps[c].items():
            core_sim.tensor(k)[:] = v
    sim.simulate()
    print("sim global_time (ns):", sim.global_time)

    class R:
        pass

    r = R()
    r.results = [{"out_h": np.asarray(sim.cores[c].tensor("out_h"))}
                 for c in range(NCORES)]
    return r


def kernel(x, edge_index, edge_attr, Wi, bi, Wq, bq, Wk, bk, Wv, bv, We,
           Wskip, bskip, Wbeta, ln_g, ln_b):
    x = np.asarray(x, np.float32)
    edge_index = np.asarray(edge_index, np.int32)
    edge_attr = np.asarray(edge_attr, np.float32)
    Wi = np.asarray(Wi, np.float32)
    bi = np.asarray(bi, np.float32)
    Wq = np.asarray(Wq, np.float32)
    bq = np.asarray(bq, np.float32)
    Wk = np.asarray(Wk, np.float32)
    bk = np.asarray(bk, np.float32)
    Wv = np.asarray(Wv, np.float32)
    bv = np.asarray(bv, np.float32)
    We = np.asarray(We, np.float32)
    Wskip = np.asarray(Wskip, np.float32)
    bskip = np.asarray(bskip, np.float32)
    Wbeta = np.asarray(Wbeta, np.float32)
    ln_g = np.asarray(ln_g, np.float32)
    ln_b = np.asarray(ln_b, np.float32)

    xT, idxw, eav, msk, DgA, DgB, perm = _preprocess(x, edge_index, edge_attr)

    WeV = We[:, 0, :]                                     # [L, HID]
    wA = Wbeta[:, 0:HID, 0] + Wbeta[:, 2 * HID:3 * HID, 0]
    wB = Wbeta[:, HID:2 * HID, 0] - Wbeta[:, 2 * HID:3 * HID, 0]

    def rep(a, dt=np.float32):  # [L, HID] -> [L, P, HID]
        return np.broadcast_to(a[:, None, :], (L, P, HID)).astype(dt).copy()

    import ml_dtypes
    common = {
        "Wi": Wi, "bi": bi[None, :],
        "Wq": Wq, "Wk": Wk, "Wv": Wv, "Ws": Wskip,
        "bq": bq[:, None, :], "bk": bk[:, None, :],
        "bv": bv[:, None, :], "bs": bskip[:, None, :],
        "WeR": rep(WeV), "We16": rep(WeV, ml_dtypes.bfloat16),
        "wAR": rep(wA), "wBR": rep(wB),
        "lgR": rep(ln_g), "lbR": rep(ln_b),
    }
    in_maps = []
    for c in range(NCORES):
        m = dict(common)
        m["xT"] = xT[c]
        m["idxw"] = idxw[c]
        m["eav"] = eav[c]
        m["msk"] = msk[c]
        in_maps.append(m)

    nc = _build(DgA, DgB)
    global _last_in_maps
    _last_in_maps = in_maps
    if os.environ.get("BASS_KERNEL_SIM") == "1":
        res = _run_sim(nc, in_maps)
    else:
        res = bass_utils.run_bass_kernel_spmd(nc, in_maps,
                                              core_ids=list(range(NCORES)))
    out = np.empty((N, HID), np.float32)
    for c in range(NCORES):
        out[c * NLOC + perm[c]] = res.results[c]["out_h"][:NLOC]
    return out
